# revision 9
# baseline (speedup 1.0000x reference)
"""Mixtral-style MoE kernel for 8 Trainium2 NeuronCores.

Sharding: pure expert-parallel (one expert per core).  The host computes the
router (logits -> softmax -> top-2 -> renormalize) in float64 -- numerically
safe because the smallest top-2/3rd-place logit gap on any token is ~1e-4
while fp32 matmul noise is ~2e-6 -- gathers each expert's tokens, and ships
them to that expert's core already transposed and cast to bf16.  Each core
then runs a dense SwiGLU MLP for its expert:

    hT[i, t]  = silu(w1 x)[i, t] * (w3 x)[i, t]      (GEMM1, bf16, PSUM fp32)
    yT[h, t]  = sum_i w2[h, i] hT[i, t]              (GEMM2, bf16, PSUM fp32)

Activations stay transposed ([feature, token]) through the whole pipeline so
no on-device transposes are needed, and there are no collectives: the host
scatter-adds the per-expert outputs (scaled by the routing weights) into the
final [T, H] output.

Weights are pre-swizzled on the host into DMA-friendly layouts (>=1KB
contiguous lines per SBUF partition) and streamed through double-buffered
SBUF pools, overlapping the ~150us of weight DMA under the ~355us of PE time.
"""
import sys

sys.path.insert(0, "/opt/trn_rl_repo")

import math

import numpy as np

T, H, E, I, TOPK = 2048, 2048, 8, 4096, 2
N_CORES = 8
KC = H // 128            # 16 contraction chunks for GEMM1
NI = I // 128            # 32 intermediate chunks (= GEMM2 contraction chunks)
NH = H // 128            # 16 output chunks for GEMM2
W13_GROUPS = 8           # stream w13 in 8 groups of 8 chunks (4 gate/up pairs)
W2_GROUPS = 4            # stream w2 in 4 groups of 4 output chunks

_CACHE = {}


def _col_tiles(cap):
    """Split cap token columns into <=512-wide tiles (PSUM bank limit)."""
    nct = max(1, math.ceil(cap / 512))
    base = cap // nct
    rem = cap - base * nct
    tiles, c0 = [], 0
    for i in range(nct):
        w = base + (1 if i < rem else 0)
        tiles.append((c0, w))
        c0 += w
    return tiles


def _build_program(cap):
    import concourse.bass as bass  # noqa: F401  (registers bass ops)
    import concourse.bacc as bacc
    import concourse.mybir as mybir
    import concourse.tile as tile

    F32, BF16 = mybir.dt.float32, mybir.dt.bfloat16
    AF = mybir.ActivationFunctionType

    nc = bacc.Bacc("TRN2", target_bir_lowering=False, debug=False,
                   enable_asserts=False, num_devices=1)

    # [kc, k, col] : token columns, transposed, bf16
    xd = nc.dram_tensor("xt", [KC, 128, cap], BF16, kind="ExternalInput")
    # [g, kc, k, (j_local, m)] : w13^T tiles, chunk order g0,u0,g1,u1,...
    w13d = nc.dram_tensor("w13", [W13_GROUPS, KC, 128, 8 * 128], BF16,
                          kind="ExternalInput")
    # [g2, kc2, k, (j2_local, m)] : w2^T tiles
    w2d = nc.dram_tensor("w2", [W2_GROUPS, NI, 128, 4 * 128], BF16,
                         kind="ExternalInput")
    # [j2, k, col] : output, transposed, fp32
    yd = nc.dram_tensor("y", [NH, 128, cap], F32, kind="ExternalOutput")

    tiles = _col_tiles(cap)

    with tile.TileContext(nc) as tc:
        with tc.tile_pool(name="xp", bufs=1) as xp, \
             tc.tile_pool(name="w13p", bufs=2) as w13p, \
             tc.tile_pool(name="w2p", bufs=2) as w2p, \
             tc.tile_pool(name="hp", bufs=1) as hp, \
             tc.tile_pool(name="sp", bufs=2) as sp, \
             tc.tile_pool(name="yp", bufs=6) as yp, \
             tc.tile_pool(name="ps", bufs=2, space="PSUM") as ps:

            # x is split into two kc-half tiles so the first GEMM1 chain can
            # start as soon as the first half (+ first w13 half) has landed.
            xh = [xp.tile([128, KC // 2, cap], BF16, tag=f"xh{h}",
                          name=f"xh{h}") for h in range(2)]
            xr = xd.rearrange("(h kc) k c -> h k kc c", h=2)
            nc.sync.dma_start(xh[0][:], xr[0])

            ht = [hp.tile([128, NI, tw], BF16, tag=f"ht{ct}", name=f"ht{ct}")
                  for ct, (_, tw) in enumerate(tiles)]

            w13r = w13d.rearrange("g (h kc) k jm -> g h k kc jm", h=2)

            def _xs(kc, c0, tw):
                return xh[kc // (KC // 2)][:, kc % (KC // 2), c0:c0 + tw]

            # ---------------- GEMM1 + SwiGLU ----------------
            w13h = {}
            w13h[(0, 0)] = w13p.tile([128, KC // 2, 8 * 128], BF16,
                                     tag="w13a", name="w13a")
            nc.sync.dma_start(w13h[(0, 0)][:], w13r[0, 0])
            nc.sync.dma_start(xh[1][:], xr[1])
            w13h[(0, 1)] = w13p.tile([128, KC // 2, 8 * 128], BF16,
                                     tag="w13b", name="w13b")
            nc.sync.dma_start(w13h[(0, 1)][:], w13r[0, 1])
            for g in range(W13_GROUPS):
                if g + 1 < W13_GROUPS:
                    for h in range(2):
                        t = w13p.tile([128, KC // 2, 8 * 128], BF16,
                                      tag=f"w13{'ab'[h]}", name=f"w13{'ab'[h]}")
                        nc.sync.dma_start(t[:], w13r[g + 1, h])
                        w13h[(g + 1, h)] = t
                for ct, (c0, tw) in enumerate(tiles):
                    for lp in range(4):
                        i = g * 4 + lp
                        pg = ps.tile([128, tw], F32, space="PSUM", tag="pg")
                        pu = ps.tile([128, tw], F32, space="PSUM", tag="pu")
                        for kc in range(KC):
                            nc.tensor.matmul(
                                pg[:],
                                lhsT=w13h[(g, kc // 8)][
                                    :, kc % 8, 2 * lp * 128:
                                    (2 * lp + 1) * 128],
                                rhs=_xs(kc, c0, tw),
                                start=(kc == 0), stop=(kc == KC - 1))
                        for kc in range(KC):
                            nc.tensor.matmul(
                                pu[:],
                                lhsT=w13h[(g, kc // 8)][
                                    :, kc % 8, (2 * lp + 1) * 128:
                                    (2 * lp + 2) * 128],
                                rhs=_xs(kc, c0, tw),
                                start=(kc == 0), stop=(kc == KC - 1))
                        st = sp.tile([128, tw], F32, tag="st")
                        nc.scalar.activation(st[:], pg[:], AF.Silu)
                        nc.vector.tensor_tensor(
                            out=ht[ct][:, i, :], in0=st[:], in1=pu[:],
                            op=mybir.AluOpType.mult)

            # ---------------- GEMM2 ----------------
            # Emit group g2+1's load BEFORE group g2's compute+stores so the
            # load DMA never queues behind this group's y-store DMAs.
            w2r = w2d.rearrange("g kc k jm -> g k kc jm")
            w2tiles = {}
            w2tiles[0] = w2p.tile([128, NI, 4 * 128], BF16, tag="w2",
                                  name="w2t")
            nc.sync.dma_start(w2tiles[0][:], w2r[0])
            for g2 in range(W2_GROUPS):
                if g2 + 1 < W2_GROUPS:
                    t = w2p.tile([128, NI, 4 * 128], BF16, tag="w2",
                                 name="w2t")
                    nc.sync.dma_start(t[:], w2r[g2 + 1])
                    w2tiles[g2 + 1] = t
                w2t = w2tiles.pop(g2)
                for ct, (c0, tw) in enumerate(tiles):
                    for j2l in range(4):
                        j2 = g2 * 4 + j2l
                        po = ps.tile([128, tw], F32, space="PSUM", tag="po")
                        for kc2 in range(NI):
                            nc.tensor.matmul(
                                po[:],
                                lhsT=w2t[:, kc2, j2l * 128:(j2l + 1) * 128],
                                rhs=ht[ct][:, kc2, :],
                                start=(kc2 == 0), stop=(kc2 == NI - 1))
                        yt = yp.tile([128, tw], F32, tag="yt")
                        nc.vector.tensor_copy(yt[:], po[:])
                        # y-stores issue from the ACT engine's HWDGE queue so
                        # they never queue behind a parked w2-load on SP.SEQ
                        # (head-of-line blocking stalls PSUM recycling).
                        nc.scalar.dma_start(yd[j2, :, c0:c0 + tw], yt[:])

    nc.compile()
    return nc


def _route(hidden_states, gate_weight):
    """Host router: exact reference math in float64."""
    logits = (hidden_states.astype(np.float64)
              @ gate_weight.astype(np.float64).T)          # [T, E]
    p = np.exp(logits - logits.max(axis=1, keepdims=True))
    p /= p.sum(axis=1, keepdims=True)
    top2 = np.argsort(-logits, axis=1)[:, :TOPK]           # [T, 2]
    tw = np.take_along_axis(p, top2, axis=1)
    tw /= tw.sum(axis=1, keepdims=True)                    # renormalize
    ids, gates = [], []
    for e in range(E):
        tok, rank = np.nonzero(top2 == e)
        ids.append(tok)
        gates.append(tw[tok, rank])
    return ids, gates


def _prepare(hidden_states, gate_weight, w13_weight, w2_weight):
    """Host routing + gather + weight swizzle. Returns (cap, in_maps, ids,
    gates)."""
    import concourse.mybir as mybir
    bf16 = mybir.dt.np(mybir.dt.bfloat16)

    ids, gates = _route(hidden_states, gate_weight)
    cap = max(4, ((max(len(t) for t in ids) + 3) // 4) * 4)

    in_maps = []
    for e in range(E):
        tok = ids[e]
        xt = np.zeros((H, cap), dtype=bf16)
        xt[:, :len(tok)] = hidden_states[tok].astype(bf16).T
        xt = np.ascontiguousarray(xt.reshape(KC, 128, cap))

        w = w13_weight[e]                                  # [2I, H]
        ga = w[:I].reshape(NI, 128, H)
        up = w[I:].reshape(NI, 128, H)
        inter = np.stack([ga, up], axis=1).reshape(2 * NI, 128, H)
        # [g, jl, m, kc, k] -> [g, kc, k, jl, m]
        a = inter.reshape(W13_GROUPS, 8, 128, KC, 128)
        w13t = np.ascontiguousarray(
            a.transpose(0, 3, 4, 1, 2)).astype(bf16).reshape(
                W13_GROUPS, KC, 128, 8 * 128)

        w2t = w2_weight[e].T                               # [I, H]
        b = w2t.reshape(NI, 128, NH, 128)                  # [kc2, k, j2, m]
        c = b.transpose(2, 0, 1, 3).reshape(W2_GROUPS, 4, NI, 128, 128)
        w2p = np.ascontiguousarray(
            c.transpose(0, 2, 3, 1, 4)).astype(bf16).reshape(
                W2_GROUPS, NI, 128, 4 * 128)

        in_maps.append(dict(xt=xt, w13=w13t, w2=w2p))
    return cap, in_maps, ids, gates


def _combine(results, ids, gates, cap):
    out = np.zeros((T, H), dtype=np.float32)
    for e in range(E):
        y = np.asarray(results[e]["y"], np.float32).reshape(H, cap)
        out[ids[e]] += gates[e][:, None] * y[:, :len(ids[e])].T
    return out


def kernel(hidden_states, gate_weight, w13_weight, w2_weight, top_k):
    assert int(top_k) == TOPK
    hidden_states = np.asarray(hidden_states, dtype=np.float32)
    gate_weight = np.asarray(gate_weight, dtype=np.float32)
    w13_weight = np.asarray(w13_weight, dtype=np.float32)
    w2_weight = np.asarray(w2_weight, dtype=np.float32)

    cap, in_maps, ids, gates = _prepare(
        hidden_states, gate_weight, w13_weight, w2_weight)
    if cap not in _CACHE:
        _CACHE[cap] = _build_program(cap)
    nc = _CACHE[cap]

    from concourse.bass_utils import run_bass_kernel_spmd
    res = run_bass_kernel_spmd(nc, in_maps, core_ids=list(range(N_CORES)),
                               trace=False)
    return _combine(res.results, ids, gates, cap)


# revision 15
# speedup vs baseline: 1.1770x; 1.1770x over previous
"""Mixtral-style MoE kernel for 8 Trainium2 NeuronCores.

Sharding: pure expert-parallel (one expert per core).  The host computes the
router (logits -> softmax -> top-2 -> renormalize) in float64 -- numerically
safe because the smallest top-2/3rd-place logit gap on any token is ~1e-4
while fp32 matmul noise is ~2e-6 -- gathers each expert's tokens, and ships
them to that expert's core already transposed and cast to bf16.  Each core
then runs a dense SwiGLU MLP for its expert:

    hT[i, t]  = silu(w1 x)[i, t] * (w3 x)[i, t]      (GEMM1, bf16, PSUM fp32)
    yT[h, t]  = sum_i w2[h, i] hT[i, t]              (GEMM2, bf16, PSUM fp32)

Activations stay transposed ([feature, token]) through the whole pipeline so
no on-device transposes are needed, and there are no collectives: the host
scatter-adds the per-expert outputs (scaled by the routing weights) into the
final [T, H] output.

Weights are pre-swizzled on the host into DMA-friendly layouts (>=1KB
contiguous lines per SBUF partition) and streamed through double-buffered
SBUF pools, overlapping the ~150us of weight DMA under the ~355us of PE time.
"""
import sys

sys.path.insert(0, "/opt/trn_rl_repo")

import math

import numpy as np

T, H, E, I, TOPK = 2048, 2048, 8, 4096, 2
N_CORES = 8
KC = H // 128            # 16 contraction chunks for GEMM1
NI = I // 128            # 32 intermediate chunks (= GEMM2 contraction chunks)
NH = H // 128            # 16 output chunks for GEMM2
W13_GROUPS = 8           # stream w13 in 8 groups of 8 chunks (4 gate/up pairs)
W2_GROUPS = 4            # stream w2 in 4 groups of 4 output chunks

_CACHE = {}


def _col_tiles(cap):
    """Split cap token columns into <=512-wide tiles (PSUM bank limit)."""
    nct = max(1, math.ceil(cap / 512))
    base = cap // nct
    rem = cap - base * nct
    tiles, c0 = [], 0
    for i in range(nct):
        w = base + (1 if i < rem else 0)
        tiles.append((c0, w))
        c0 += w
    return tiles


def _build_program(cap):
    import concourse.bass as bass  # noqa: F401  (registers bass ops)
    import concourse.bacc as bacc
    import concourse.mybir as mybir
    import concourse.tile as tile

    F32, BF16 = mybir.dt.float32, mybir.dt.bfloat16
    AF = mybir.ActivationFunctionType

    nc = bacc.Bacc("TRN2", target_bir_lowering=False, debug=False,
                   enable_asserts=False, num_devices=1)

    # [kc, k, col] : token columns, transposed, bf16
    xd = nc.dram_tensor("xt", [KC, 128, cap], BF16, kind="ExternalInput")
    # [g, kc, k, (j_local, m)] : w13^T tiles, chunk order g0,u0,g1,u1,...
    w13d = nc.dram_tensor("w13", [W13_GROUPS, KC, 128, 8 * 128], BF16,
                          kind="ExternalInput")
    # [g2, kc2, k, (j2_local, m)] : w2^T tiles
    w2d = nc.dram_tensor("w2", [W2_GROUPS, NI, 128, 4 * 128], BF16,
                         kind="ExternalInput")
    # [j2, k, col] : output, transposed, fp32
    yd = nc.dram_tensor("y", [NH, 128, cap], F32, kind="ExternalOutput")

    tiles = _col_tiles(cap)

    with tile.TileContext(nc) as tc:
        with tc.tile_pool(name="xp", bufs=1) as xp, \
             tc.tile_pool(name="w13p", bufs=2) as w13p, \
             tc.tile_pool(name="w2p", bufs=2) as w2p, \
             tc.tile_pool(name="hp", bufs=1) as hp, \
             tc.tile_pool(name="sp", bufs=2) as sp, \
             tc.tile_pool(name="yp", bufs=6) as yp, \
             tc.tile_pool(name="ps", bufs=2, space="PSUM") as ps:

            # x is split into two kc-half tiles so the first GEMM1 chain can
            # start as soon as the first half (+ first w13 half) has landed.
            xh = [xp.tile([128, KC // 2, cap], BF16, tag=f"xh{h}",
                          name=f"xh{h}") for h in range(2)]
            xr = xd.rearrange("(h kc) k c -> h k kc c", h=2)
            nc.sync.dma_start(xh[0][:], xr[0])

            ht = [hp.tile([128, NI, tw], BF16, tag=f"ht{ct}", name=f"ht{ct}")
                  for ct, (_, tw) in enumerate(tiles)]

            w13r = w13d.rearrange("g (h kc) k jm -> g h k kc jm", h=2)

            def _xs(kc, c0, tw):
                return xh[kc // (KC // 2)][:, kc % (KC // 2), c0:c0 + tw]

            # ---------------- GEMM1 + SwiGLU ----------------
            w13h = {}
            w13h[(0, 0)] = w13p.tile([128, KC // 2, 8 * 128], BF16,
                                     tag="w13a", name="w13a")
            nc.sync.dma_start(w13h[(0, 0)][:], w13r[0, 0])
            nc.sync.dma_start(xh[1][:], xr[1])
            w13h[(0, 1)] = w13p.tile([128, KC // 2, 8 * 128], BF16,
                                     tag="w13b", name="w13b")
            nc.sync.dma_start(w13h[(0, 1)][:], w13r[0, 1])
            for g in range(W13_GROUPS):
                if g + 1 < W13_GROUPS:
                    for h in range(2):
                        t = w13p.tile([128, KC // 2, 8 * 128], BF16,
                                      tag=f"w13{'ab'[h]}", name=f"w13{'ab'[h]}")
                        nc.sync.dma_start(t[:], w13r[g + 1, h])
                        w13h[(g + 1, h)] = t
                for ct, (c0, tw) in enumerate(tiles):
                    for lp in range(4):
                        i = g * 4 + lp
                        pg = ps.tile([128, tw], F32, space="PSUM", tag="pg")
                        pu = ps.tile([128, tw], F32, space="PSUM", tag="pu")
                        for kc in range(KC):
                            nc.tensor.matmul(
                                pg[:],
                                lhsT=w13h[(g, kc // 8)][
                                    :, kc % 8, 2 * lp * 128:
                                    (2 * lp + 1) * 128],
                                rhs=_xs(kc, c0, tw),
                                start=(kc == 0), stop=(kc == KC - 1))
                        for kc in range(KC):
                            nc.tensor.matmul(
                                pu[:],
                                lhsT=w13h[(g, kc // 8)][
                                    :, kc % 8, (2 * lp + 1) * 128:
                                    (2 * lp + 2) * 128],
                                rhs=_xs(kc, c0, tw),
                                start=(kc == 0), stop=(kc == KC - 1))
                        st = sp.tile([128, tw], F32, tag="st")
                        nc.scalar.activation(st[:], pg[:], AF.Silu)
                        nc.vector.tensor_tensor(
                            out=ht[ct][:, i, :], in0=st[:], in1=pu[:],
                            op=mybir.AluOpType.mult)

            # ---------------- GEMM2 ----------------
            # Emit group g2+1's load BEFORE group g2's compute+stores so the
            # load DMA never queues behind this group's y-store DMAs.
            w2r = w2d.rearrange("g kc k jm -> g k kc jm")
            w2tiles = {}
            w2tiles[0] = w2p.tile([128, NI, 4 * 128], BF16, tag="w2",
                                  name="w2t")
            nc.sync.dma_start(w2tiles[0][:], w2r[0])
            for g2 in range(W2_GROUPS):
                if g2 + 1 < W2_GROUPS:
                    t = w2p.tile([128, NI, 4 * 128], BF16, tag="w2",
                                 name="w2t")
                    nc.sync.dma_start(t[:], w2r[g2 + 1])
                    w2tiles[g2 + 1] = t
                w2t = w2tiles.pop(g2)
                for ct, (c0, tw) in enumerate(tiles):
                    for j2l in range(4):
                        j2 = g2 * 4 + j2l
                        po = ps.tile([128, tw], F32, space="PSUM", tag="po")
                        for kc2 in range(NI):
                            nc.tensor.matmul(
                                po[:],
                                lhsT=w2t[:, kc2, j2l * 128:(j2l + 1) * 128],
                                rhs=ht[ct][:, kc2, :],
                                start=(kc2 == 0), stop=(kc2 == NI - 1))
                        yt = yp.tile([128, tw], F32, tag="yt")
                        nc.vector.tensor_copy(yt[:], po[:])
                        # y-stores issue from the ACT engine's HWDGE queue so
                        # they never queue behind a parked w2-load on SP.SEQ
                        # (head-of-line blocking stalls PSUM recycling).
                        nc.scalar.dma_start(yd[j2, :, c0:c0 + tw], yt[:])

    nc.compile()
    return nc


def _route(hidden_states, gate_weight):
    """Host router: exact reference math in float64."""
    logits = (hidden_states.astype(np.float64)
              @ gate_weight.astype(np.float64).T)          # [T, E]
    p = np.exp(logits - logits.max(axis=1, keepdims=True))
    p /= p.sum(axis=1, keepdims=True)
    top2 = np.argsort(-logits, axis=1)[:, :TOPK]           # [T, 2]
    tw = np.take_along_axis(p, top2, axis=1)
    tw /= tw.sum(axis=1, keepdims=True)                    # renormalize
    ids, gates = [], []
    for e in range(E):
        tok, rank = np.nonzero(top2 == e)
        ids.append(tok)
        gates.append(tw[tok, rank])
    return ids, gates


def _prepare(hidden_states, gate_weight, w13_weight, w2_weight):
    """Host routing + gather + weight swizzle. Returns (cap, in_maps, ids,
    gates)."""
    import concourse.mybir as mybir
    bf16 = mybir.dt.np(mybir.dt.bfloat16)

    ids, gates = _route(hidden_states, gate_weight)
    cap = max(4, ((max(len(t) for t in ids) + 1) // 2) * 2)

    in_maps = []
    for e in range(E):
        tok = ids[e]
        xt = np.zeros((H, cap), dtype=bf16)
        xt[:, :len(tok)] = hidden_states[tok].astype(bf16).T
        xt = np.ascontiguousarray(xt.reshape(KC, 128, cap))

        w = w13_weight[e]                                  # [2I, H]
        ga = w[:I].reshape(NI, 128, H)
        up = w[I:].reshape(NI, 128, H)
        inter = np.stack([ga, up], axis=1).reshape(2 * NI, 128, H)
        # [g, jl, m, kc, k] -> [g, kc, k, jl, m]
        a = inter.reshape(W13_GROUPS, 8, 128, KC, 128)
        w13t = np.ascontiguousarray(
            a.transpose(0, 3, 4, 1, 2)).astype(bf16).reshape(
                W13_GROUPS, KC, 128, 8 * 128)

        w2t = w2_weight[e].T                               # [I, H]
        b = w2t.reshape(NI, 128, NH, 128)                  # [kc2, k, j2, m]
        c = b.transpose(2, 0, 1, 3).reshape(W2_GROUPS, 4, NI, 128, 128)
        w2p = np.ascontiguousarray(
            c.transpose(0, 2, 3, 1, 4)).astype(bf16).reshape(
                W2_GROUPS, NI, 128, 4 * 128)

        in_maps.append(dict(xt=xt, w13=w13t, w2=w2p))
    return cap, in_maps, ids, gates


def _combine(results, ids, gates, cap):
    out = np.zeros((T, H), dtype=np.float32)
    for e in range(E):
        y = np.asarray(results[e]["y"], np.float32).reshape(H, cap)
        out[ids[e]] += gates[e][:, None] * y[:, :len(ids[e])].T
    return out


def kernel(hidden_states, gate_weight, w13_weight, w2_weight, top_k):
    assert int(top_k) == TOPK
    hidden_states = np.asarray(hidden_states, dtype=np.float32)
    gate_weight = np.asarray(gate_weight, dtype=np.float32)
    w13_weight = np.asarray(w13_weight, dtype=np.float32)
    w2_weight = np.asarray(w2_weight, dtype=np.float32)

    cap, in_maps, ids, gates = _prepare(
        hidden_states, gate_weight, w13_weight, w2_weight)
    if cap not in _CACHE:
        _CACHE[cap] = _build_program(cap)
    nc = _CACHE[cap]

    from concourse.bass_utils import run_bass_kernel_spmd
    res = run_bass_kernel_spmd(nc, in_maps, core_ids=list(range(N_CORES)),
                               trace=False)
    return _combine(res.results, ids, gates, cap)
